# revision 50
# baseline (speedup 1.0000x reference)
"""GQA sparse attention (packed seqs + sliding window + RoPE) on 8 Trainium2 cores.

Sharding: tensor-parallel over heads. Each of the 8 cores owns 4 Q-heads and
their single shared KV-head (GQA groups stay intact): wq columns
[h*512:(h+1)*512], wk/wv columns [h*128:(h+1)*128], wo rows [h*512:(h+1)*512].
Every core computes a full [S, DIM] partial of the output projection; the host
sums the 8 partials.

The mask never reaches the device: seqlens [1024, 512, 512] with causal +
sliding-window 1024 reduce to block-causal over 128-blocks within each
sequence (the window can never truncate since max causal span == 1024), plus
a causal bias on the diagonal 128x128 blocks.

Per-core dataflow (all matmuls bf16 with fp32 PSUM accumulation):
  qkv:   psum[s,768] = sum_cb xT[cb,si].T @ wqkv[cb]      (weights resident)
  rope:  strided DVE ops on the psum, [s,d] layout, fp32 in / bf16 out
  qT/kT: PE transposes of the roped blocks
  scores(T): psum[sk, sq_span] = kT_blk.T @ qT[h]         (block-causal spans)
  p:     exp(scores + diag_bias) -> pT buffer, bf16       (no max subtraction:
         scores are O(5), exp is safe in fp32)
  pv:    psum[sq, 129] = sum_kj pT_blk.T @ [v_blk | ones] (sums ride along)
  out:   attn = pv[:, :128] * recip(pv[:, 128]),  transpose -> attnT
  wo:    psum[c',s] = sum_db wo[db,cp].T @ attnT[db]      -> DRAM [4096, 2048]
"""

import os

os.environ.setdefault("JAX_PLATFORMS", "axon")

import numpy as np

import concourse.bass as bass
import concourse.mybir as mybir
import concourse.tile as tile
from concourse import bacc
from concourse.bass_utils import run_bass_kernel_spmd

# ---- problem constants (hardcoded per harness contract) ----
DIM = 4096
N_HEADS = 32
N_KV_HEADS = 8
HEAD_DIM = 128
SEQLENS = [1024, 512, 512]
S = 2048
N_CORES = 8
HPC = N_HEADS // N_CORES          # q heads per core = 4
QW = HPC * HEAD_DIM               # per-core q width = 512
KW = HEAD_DIM                     # per-core k/v width = 128
B = 128                           # block size
NSB = S // B                      # 16 seq blocks
NCB = DIM // B                    # 32 contraction blocks
SEQ_BLOCKS = []                   # [(start_blk, end_blk)] per packed sequence
_b = 0
for _l in SEQLENS:
    SEQ_BLOCKS.append((_b, _b + _l // B))
    _b += _l // B

# pT buffer layout: for each kj, columns [offs[kj] : offs[kj]+span(kj)) hold
# p.T for queries sq in [kj*B, seq_end)
_SPANS = {}
_OFFS = {}
_off = 0
for _s0, _s1 in SEQ_BLOCKS:
    for _kj in range(_s0, _s1):
        _SPANS[_kj] = (_s1 - _kj) * B
        _OFFS[_kj] = _off
        _off += _SPANS[_kj]
PT_COLS = _off                    # 7168

F32 = mybir.dt.float32
BF16 = mybir.dt.bfloat16

_PROGRAM = None


def _build_program():
    nc = bacc.Bacc(trn_type="TRN2")

    xt_h = nc.declare_dram_parameter("xt", [NSB, B, DIM], BF16, isOutput=False)
    wqkv_h = nc.declare_dram_parameter("wqkv", [DIM, QW + 2 * KW], BF16, isOutput=False)
    wo_h = nc.declare_dram_parameter("wo", [QW, DIM], BF16, isOutput=False)
    cos_h = nc.declare_dram_parameter("cosr", [NSB, B, 2 * HEAD_DIM], F32, isOutput=False)
    sin_h = nc.declare_dram_parameter("sinr", [NSB, B, 2 * HEAD_DIM], F32, isOutput=False)
    dmask_h = nc.declare_dram_parameter("dmask", [B, B], F32, isOutput=False)
    ident_h = nc.declare_dram_parameter("ident", [B, B], BF16, isOutput=False)
    out_h = nc.declare_dram_parameter("outp", [DIM, S], BF16, isOutput=True)

    W768 = QW + 2 * KW  # 768
    Exp = mybir.ActivationFunctionType.Exp

    with tile.TileContext(nc) as tc:
        with (
            tc.tile_pool(name="consts", bufs=1) as cpool,
            tc.tile_pool(name="big", bufs=1) as bigp,
            tc.tile_pool(name="persist", bufs=1) as pers,
            tc.tile_pool(name="roam", bufs=4) as roam,
            tc.tile_pool(name="work", bufs=3) as work,
            tc.tile_pool(name="psum", bufs=2, space="PSUM") as psum,
        ):
            # first x block before the big weight DMAs so PE starts early
            xt0_t = work.tile([B, DIM], BF16, tag="xt", bufs=3)
            nc.sync.dma_start(out=xt0_t[:], in_=xt_h[0])

            # ---- resident tensors ----
            # interleave early x blocks into the weight stream so si=1..3
            # can start before the full wqkv has landed
            early_xt = {}
            wqkv_sb = bigp.tile([B, NCB * W768], BF16, tag="big")
            for cb in range(NCB):
                nc.sync.dma_start(
                    out=wqkv_sb[:, cb * W768:(cb + 1) * W768],
                    in_=wqkv_h[cb * B:(cb + 1) * B, :],
                )
                if cb in (7, 15, 23):
                    si_pre = cb // 8 + 1
                    t = work.tile([B, DIM], BF16, tag="xt", bufs=3, name=f"xtp{si_pre}")
                    nc.sync.dma_start(out=t[:], in_=xt_h[si_pre])
                    early_xt[si_pre] = t

            # ---- constants (after weights: nothing needs them until rope) ----
            ident_sb = cpool.tile([B, B], BF16)
            nc.sync.dma_start(out=ident_sb[:], in_=ident_h[:])
            dmask_sb = cpool.tile([B, B], F32)
            nc.sync.dma_start(out=dmask_sb[:], in_=dmask_h[:])

            attnT_sb = pers.tile([B, HPC * S], BF16)   # per head h: cols [h*S, (h+1)*S)
            qT_sb = pers.tile([B, HPC * S], BF16)      # per head h: cols [h*S, (h+1)*S)
            kT_sb = pers.tile([B, S], BF16)
            vaug_sb = pers.tile([B, NSB * 129], BF16)  # per kj: [v_blk | ones]

            # scores + exp for one (head, kj) block-row
            pTs = []

            def _scores(h, kj, s1, chunked=False):
                pT = pTs[h]
                span = (s1 - kj) * B
                if chunked:
                    # phase-A-overlapped variant: 512-col chunks through tag B
                    # (PV's tag, idle during phase A) so the qkv psum pipeline
                    # in tag A is never paced by exp latency
                    for part in range(0, span, 512):
                        n = min(512, span - part)
                        ps_c = psum.tile([B, 512], F32, tag="B", bufs=2,
                                         name="ps_c")
                        nc.tensor.matmul(
                            ps_c[:, 0:n],
                            kT_sb[:, kj * B:(kj + 1) * B],
                            qT_sb[:, h * S + kj * B + part:
                                  h * S + kj * B + part + n],
                            start=True, stop=True,
                        )
                        if part == 0:
                            nc.vector.tensor_add(
                                ps_c[:, 0:B], ps_c[:, 0:B], dmask_sb[:]
                            )
                        nc.scalar.activation(
                            pT[:, _OFFS[kj] + part:_OFFS[kj] + part + n],
                            ps_c[:, 0:n], Exp
                        )
                    return
                ps_sc = psum.tile([B, 1024], F32, tag="A", bufs=2, name="ps_sc")
                for part in range(0, span, 512):
                    n = min(512, span - part)
                    nc.tensor.matmul(
                        ps_sc[:, part:part + n],
                        kT_sb[:, kj * B:(kj + 1) * B],
                        qT_sb[:, h * S + kj * B + part:
                              h * S + kj * B + part + n],
                        start=True, stop=True,
                    )
                # causal bias on the diagonal block
                nc.vector.tensor_add(ps_sc[:, 0:B], ps_sc[:, 0:B], dmask_sb[:])
                nc.scalar.activation(
                    pT[:, _OFFS[kj]:_OFFS[kj] + span], ps_sc[:, 0:span], Exp
                )

            # seq0/seq1 score groups overlap the back half of the qkv phase
            # (their qT/kT inputs are complete by then); seq2 runs after
            chains_done = set()
            sched = {si: [] for si in range(NSB)}
            for kj in range(0, 8):
                sched[8 + kj] = [(h, kj, 8) for h in range(HPC)]
            for kj, si in ((8, 13), (9, 14), (10, 15), (11, 15)):
                sched[si] += [(h, kj, 12) for h in range(HPC)]
            done = set()

            def _chains(scol, tp_tag="C"):
                for qi in range(scol * 4, scol * 4 + 4):
                    if qi in chains_done:
                        continue
                    _chain_qi(qi, tp_tag)

            def _chain_qi(qi, tp_tag="C", copy_act=False):
                    chains_done.add(qi)
                    s0, s1 = next(b for b in SEQ_BLOCKS if b[0] <= qi < b[1])
                    for h in range(HPC):
                        pT = pTs[h]
                        ps_pv = psum.tile([B, 129], F32, tag="B", bufs=2)
                        for kj in range(s0, qi + 1):
                            lhsT = pT[:, _OFFS[kj] + (qi - kj) * B:
                                      _OFFS[kj] + (qi - kj + 1) * B]
                            nc.tensor.matmul(
                                ps_pv[:], lhsT,
                                vaug_sb[:, kj * 129:(kj + 1) * 129],
                                start=(kj == s0), stop=(kj == qi),
                            )
                        pv_sb = work.tile([B, 129], F32, tag="pv", bufs=8)
                        nc.vector.tensor_copy(pv_sb[:], ps_pv[:])
                        rc = work.tile([B, 1], F32, tag="rc", bufs=8)
                        nc.vector.reciprocal(rc[:], pv_sb[:, 128:129])
                        at = work.tile([B, B], BF16, tag="at", bufs=8)
                        nc.vector.tensor_scalar_mul(at[:], pv_sb[:, 0:B], rc[:])
                        tp = psum.tile([B, B], BF16, tag=tp_tag, bufs=2)
                        nc.tensor.transpose(tp[:], at[:], ident_sb[:])
                        dst = attnT_sb[:, h * S + qi * B:h * S + (qi + 1) * B]
                        if copy_act:
                            nc.scalar.copy(dst, tp[:])
                        else:
                            nc.vector.tensor_copy(dst, tp[:])

            def _wo(scol):
                for cp in range(NCB):
                    pso = psum.tile([B, 512], F32, tag="C", bufs=2)
                    for db in range(HPC):
                        nc.tensor.matmul(
                            pso[:],
                            wo_sb[:, db * DIM + cp * B:db * DIM + (cp + 1) * B],
                            attnT_sb[:, db * S + scol * 512:db * S + (scol + 1) * 512],
                            start=(db == 0), stop=(db == HPC - 1),
                        )
                    ot = work.tile([B, 512], BF16, tag="ot", bufs=8)
                    if cp % 2 == 0:
                        nc.scalar.copy(ot[:], pso[:])
                    else:
                        nc.vector.tensor_copy(ot[:], pso[:])
                    nc.sync.dma_start(
                        out=out_h[cp * B:(cp + 1) * B, scol * 512:(scol + 1) * 512],
                        in_=ot[:],
                    )

            # =========== Phase A: qkv projection + rope + transposes ===========
            for si in range(NSB):
                if si == 0:
                    xt_t = xt0_t
                elif si in early_xt:
                    xt_t = early_xt[si]
                else:
                    xt_t = work.tile([B, DIM], BF16, tag="xt", bufs=3)
                    nc.sync.dma_start(out=xt_t[:], in_=xt_h[si])
                # small rotating cos/sin tiles (dead after rope of this si)
                c_t = work.tile([B, 256], F32, tag="cs", bufs=3)
                nc.sync.dma_start(out=c_t[:], in_=cos_h[si])
                s_t = work.tile([B, 256], F32, tag="sn", bufs=3)
                nc.sync.dma_start(out=s_t[:], in_=sin_h[si])
                ps = psum.tile([B, W768], F32, tag="A", bufs=2)
                psQ = ps[:, 0:512]
                psKV = ps[:, 512:768]
                for cb in range(NCB):
                    lhsT = xt_t[:, cb * B:(cb + 1) * B]
                    nc.tensor.matmul(
                        psQ, lhsT, wqkv_sb[:, cb * W768:cb * W768 + 512],
                        start=(cb == 0), stop=(cb == NCB - 1),
                    )
                    nc.tensor.matmul(
                        psKV, lhsT,
                        wqkv_sb[:, cb * W768 + 512:cb * W768 + 768],
                        start=(cb == 0), stop=(cb == NCB - 1),
                    )

                cs = c_t[:]
                sn = s_t[:]

                # rope on q: [s, d] layout, channels interleaved (even, odd)
                q_t = work.tile([B, QW], BF16, tag="q", bufs=3)
                qe, qo = ps[:, 0:QW:2], ps[:, 1:QW:2]
                t1 = work.tile([B, 256], F32, tag="t1", bufs=2)
                t2 = work.tile([B, 256], F32, tag="t2", bufs=2)
                t3 = work.tile([B, 256], F32, tag="t3", bufs=2)
                t4 = work.tile([B, 256], F32, tag="t4", bufs=2)
                nc.vector.tensor_mul(t1[:], qe, cs)
                nc.vector.tensor_mul(t2[:], qo, sn)
                nc.vector.tensor_sub(q_t[:, 0:QW:2], t1[:], t2[:])
                nc.vector.tensor_mul(t3[:], qe, sn)
                nc.vector.tensor_mul(t4[:], qo, cs)
                nc.vector.tensor_add(q_t[:, 1:QW:2], t3[:], t4[:])

                # rope on k
                k_t = work.tile([B, KW], BF16, tag="k", bufs=3)
                ke, ko = ps[:, 512:640:2], ps[:, 513:640:2]
                c64, s64 = c_t[:, 0:64], s_t[:, 0:64]
                u1 = work.tile([B, 64], F32, tag="u1", bufs=2)
                u2 = work.tile([B, 64], F32, tag="u2", bufs=2)
                u3 = work.tile([B, 64], F32, tag="u3", bufs=2)
                u4 = work.tile([B, 64], F32, tag="u4", bufs=2)
                nc.vector.tensor_mul(u1[:], ke, c64)
                nc.vector.tensor_mul(u2[:], ko, s64)
                nc.vector.tensor_sub(k_t[:, 0:KW:2], u1[:], u2[:])
                nc.vector.tensor_mul(u3[:], ke, s64)
                nc.vector.tensor_mul(u4[:], ko, c64)
                nc.vector.tensor_add(k_t[:, 1:KW:2], u3[:], u4[:])

                # v block + ones column
                nc.scalar.copy(vaug_sb[:, si * 129:si * 129 + 128], ps[:, 640:768])
                nc.vector.memset(vaug_sb[:, si * 129 + 128:si * 129 + 129], 1.0)

                # transposes: q (4 blocks) and k (1 block)
                for h in range(HPC):
                    tp = psum.tile([B, B], BF16, tag="C", bufs=2)
                    nc.tensor.transpose(tp[:], q_t[:, h * B:(h + 1) * B], ident_sb[:])
                    dst = qT_sb[:, h * S + si * B:h * S + (si + 1) * B]
                    if h % 2 == 0:
                        nc.vector.tensor_copy(dst, tp[:])
                    else:
                        nc.scalar.copy(dst, tp[:])
                ktp = psum.tile([B, B], BF16, tag="C", bufs=2)
                nc.tensor.transpose(ktp[:], k_t[:], ident_sb[:])
                nc.vector.tensor_copy(kT_sb[:, si * B:(si + 1) * B], ktp[:])

                if si == 7:
                    for h in range(HPC):
                        pT = roam.tile([B, PT_COLS], BF16, tag="roam", bufs=4,
                                       name=f"pT{h}")
                        pTs.append(pT)
                for (h, kj, s1) in sched[si]:
                    _scores(h, kj, s1, chunked=True)
                    done.add((h, kj))
                qi_sched = {12: (0, 4), 13: (1, 5, 8), 14: (2, 6, 9),
                            15: (3, 7, 10, 11)}
                for qi in qi_sched.get(si, ()):
                    _chain_qi(qi)

            # wo reuses the wqkv slot; attnT is its own tensor so chains can
            # write it before the last qkv matmul retires
            big2 = bigp.tile([B, NCB * W768], BF16, tag="big")
            wo_sb = big2[:, 0:HPC * DIM]
            for db in range(HPC):
                nc.sync.dma_start(
                    out=wo_sb[:, db * DIM:(db + 1) * DIM],
                    in_=wo_h[db * B:(db + 1) * B, :],
                )

            _B1_TODO = [
                (h, kj, s1)
                for s0, s1 in SEQ_BLOCKS
                for kj in range(s0, s1)
                for h in range(HPC)
                if (h, kj) not in done
            ]

            # ===== Phase B2: PV + normalize, interleaved with wo per scol =====
            # chains run one scol-group ahead of the output projection so the
            # dense wo matmuls overlap the latency-bound softmax chains
            # seq0/seq1 chains already ran inside phase A; seq2 scores and
            # its chains overlap the dense wo passes
            # seq2: emit each kj's scores then immediately its qi=kj chain
            # (qi needs only exps kj'<=qi), all overlapping wo(0)'s dense work
            _wo(0)
            for kj in range(12, 16):
                for h in range(HPC):
                    if (h, kj) not in done:
                        _scores(h, kj, 16)
                _chain_qi(kj, tp_tag="B", copy_act=True)
            _wo(1)
            _wo(2)
            _wo(3)

    nc.finalize()
    return nc


def get_program():
    global _PROGRAM
    if _PROGRAM is None:
        _PROGRAM = _build_program()
    return _PROGRAM


def make_in_maps(x, cos, sin, wq, wk, wv, wo):
    bf16 = np.dtype("bfloat16") if hasattr(np, "bfloat16") else None
    import ml_dtypes
    bf16 = ml_dtypes.bfloat16

    x = np.asarray(x, np.float32)
    cos = np.asarray(cos, np.float32)
    sin = np.asarray(sin, np.float32)
    wq = np.asarray(wq, np.float32)
    wk = np.asarray(wk, np.float32)
    wv = np.asarray(wv, np.float32)
    wo = np.asarray(wo, np.float32)

    # xt[si, p, cb*B + s] = x[si*B + s, cb*B + p]
    xt = np.ascontiguousarray(
        x.reshape(NSB, B, NCB, B).transpose(0, 3, 2, 1).reshape(NSB, B, DIM)
    ).astype(bf16)
    # cos/sin tiled 4x along channels (per-head repeat), blocked by si
    cosr = np.ascontiguousarray(np.tile(cos, (1, HPC)).reshape(NSB, B, 2 * HEAD_DIM))
    sinr = np.ascontiguousarray(np.tile(sin, (1, HPC)).reshape(NSB, B, 2 * HEAD_DIM))
    # diagonal-block causal bias in scoresT layout: allow sq >= sk
    i = np.arange(B)
    dmask = np.where(i[None, :] >= i[:, None], 0.0, -30000.0).astype(np.float32)
    ident = np.eye(B, dtype=np.float32).astype(bf16)

    scale = HEAD_DIM ** -0.5
    in_maps = []
    for c in range(N_CORES):
        wq_c = (wq[:, c * QW:(c + 1) * QW] * scale).astype(bf16)
        wk_c = wk[:, c * KW:(c + 1) * KW].astype(bf16)
        wv_c = wv[:, c * KW:(c + 1) * KW].astype(bf16)
        wqkv_c = np.ascontiguousarray(
            np.concatenate([wq_c, wk_c, wv_c], axis=1)
        )
        wo_c = np.ascontiguousarray(wo[c * QW:(c + 1) * QW, :]).astype(bf16)
        in_maps.append({
            "xt": xt,
            "wqkv": wqkv_c,
            "wo": wo_c,
            "cosr": cosr,
            "sinr": sinr,
            "dmask": dmask,
            "ident": ident,
        })
    return in_maps


def combine_outputs(results):
    acc = np.zeros((DIM, S), np.float32)
    for r in results:
        acc += np.asarray(r["outp"]).astype(np.float32)
    return np.ascontiguousarray(acc.T)


def kernel(x, cos, sin, mask, wq, wk, wv, wo):
    nc = get_program()
    in_maps = make_in_maps(x, cos, sin, wq, wk, wv, wo)
    res = run_bass_kernel_spmd(nc, in_maps, core_ids=list(range(N_CORES)))
    return combine_outputs(res.results)


# revision 65
# speedup vs baseline: 1.0207x; 1.0207x over previous
"""GQA sparse attention (packed seqs + sliding window + RoPE) on 8 Trainium2 cores.

Sharding: tensor-parallel over heads. Each of the 8 cores owns 4 Q-heads and
their single shared KV-head (GQA groups stay intact): wq columns
[h*512:(h+1)*512], wk/wv columns [h*128:(h+1)*128], wo rows [h*512:(h+1)*512].
Every core computes a full [S, DIM] partial of the output projection; the host
sums the 8 partials.

The mask never reaches the device: seqlens [1024, 512, 512] with causal +
sliding-window 1024 reduce to block-causal over 128-blocks within each
sequence (the window can never truncate since max causal span == 1024), plus
a causal bias on the diagonal 128x128 blocks.

Per-core dataflow (all matmuls bf16 with fp32 PSUM accumulation):
  qkv:   psum[s,768] = sum_cb xT[cb,si].T @ wqkv[cb]      (weights resident)
  rope:  strided DVE ops on the psum, [s,d] layout, fp32 in / bf16 out
  qT/kT: PE transposes of the roped blocks
  scores(T): psum[sk, sq_span] = kT_blk.T @ qT[h]         (block-causal spans)
  p:     exp(scores + diag_bias) -> pT buffer, bf16       (no max subtraction:
         scores are O(5), exp is safe in fp32)
  pv:    psum[sq, 129] = sum_kj pT_blk.T @ [v_blk | ones] (sums ride along)
  out:   attn = pv[:, :128] * recip(pv[:, 128]),  transpose -> attnT
  wo:    psum[c',s] = sum_db wo[db,cp].T @ attnT[db]      -> DRAM [4096, 2048]
"""

import os

os.environ.setdefault("JAX_PLATFORMS", "axon")

import numpy as np

import concourse.bass as bass
import concourse.mybir as mybir
import concourse.tile as tile
from concourse import bacc
from concourse.bass_utils import run_bass_kernel_spmd

# ---- problem constants (hardcoded per harness contract) ----
DIM = 4096
N_HEADS = 32
N_KV_HEADS = 8
HEAD_DIM = 128
SEQLENS = [1024, 512, 512]
S = 2048
N_CORES = 8
HPC = N_HEADS // N_CORES          # q heads per core = 4
QW = HPC * HEAD_DIM               # per-core q width = 512
KW = HEAD_DIM                     # per-core k/v width = 128
B = 128                           # block size
NSB = S // B                      # 16 seq blocks
NCB = DIM // B                    # 32 contraction blocks
SEQ_BLOCKS = []                   # [(start_blk, end_blk)] per packed sequence
_b = 0
for _l in SEQLENS:
    SEQ_BLOCKS.append((_b, _b + _l // B))
    _b += _l // B

# pT buffer layout: for each kj, columns [offs[kj] : offs[kj]+span(kj)) hold
# p.T for queries sq in [kj*B, seq_end)
_SPANS = {}
_OFFS = {}
_off = 0
for _s0, _s1 in SEQ_BLOCKS:
    for _kj in range(_s0, _s1):
        _SPANS[_kj] = (_s1 - _kj) * B
        _OFFS[_kj] = _off
        _off += _SPANS[_kj]
PT_COLS = _off                    # 7168

F32 = mybir.dt.float32
BF16 = mybir.dt.bfloat16

_PROGRAM = None


def _build_program():
    nc = bacc.Bacc(trn_type="TRN2")

    xt_h = nc.declare_dram_parameter("xt", [NSB, B, DIM], BF16, isOutput=False)
    wqkv_h = nc.declare_dram_parameter("wqkv", [DIM, QW + 2 * KW], BF16, isOutput=False)
    wo_h = nc.declare_dram_parameter("wo", [QW, DIM], BF16, isOutput=False)
    cos_h = nc.declare_dram_parameter("cosr", [NSB, B, 2 * HEAD_DIM], F32, isOutput=False)
    sin_h = nc.declare_dram_parameter("sinr", [NSB, B, 2 * HEAD_DIM], F32, isOutput=False)
    dmask_h = nc.declare_dram_parameter("dmask", [B, B], F32, isOutput=False)
    ident_h = nc.declare_dram_parameter("ident", [B, B], BF16, isOutput=False)
    out_h = nc.declare_dram_parameter("outp", [DIM, S], BF16, isOutput=True)

    W768 = QW + 2 * KW  # 768
    Exp = mybir.ActivationFunctionType.Exp

    with tile.TileContext(nc) as tc:
        with (
            tc.tile_pool(name="consts", bufs=1) as cpool,
            tc.tile_pool(name="big", bufs=1) as bigp,
            tc.tile_pool(name="persist", bufs=1) as pers,
            tc.tile_pool(name="roam", bufs=4) as roam,
            tc.tile_pool(name="work", bufs=3) as work,
            tc.tile_pool(name="psum", bufs=2, space="PSUM") as psum,
        ):
            # first x block before the big weight DMAs so PE starts early
            xt0_t = work.tile([B, DIM], BF16, tag="xt", bufs=3)
            nc.sync.dma_start(out=xt0_t[:], in_=xt_h[0])

            # ---- resident tensors ----
            # interleave early x blocks into the weight stream so si=1..3
            # can start before the full wqkv has landed
            early_xt = {}
            wqkv_sb = bigp.tile([B, NCB * W768], BF16, tag="big")
            for cb in range(NCB):
                nc.sync.dma_start(
                    out=wqkv_sb[:, cb * W768:(cb + 1) * W768],
                    in_=wqkv_h[cb * B:(cb + 1) * B, :],
                )
                if cb in (7, 15, 23):
                    si_pre = cb // 8 + 1
                    t = work.tile([B, DIM], BF16, tag="xt", bufs=3, name=f"xtp{si_pre}")
                    nc.sync.dma_start(out=t[:], in_=xt_h[si_pre])
                    early_xt[si_pre] = t

            # ---- constants (after weights: nothing needs them until rope) ----
            ident_sb = cpool.tile([B, B], BF16)
            nc.sync.dma_start(out=ident_sb[:], in_=ident_h[:])
            dmask_sb = cpool.tile([B, B], F32)
            nc.sync.dma_start(out=dmask_sb[:], in_=dmask_h[:])

            attnT_sb = pers.tile([B, HPC * S], BF16)   # per head h: cols [h*S, (h+1)*S)
            qT_sb = pers.tile([B, HPC * S], BF16)      # per head h: cols [h*S, (h+1)*S)
            kT_sb = pers.tile([B, S], BF16)
            vaug_sb = pers.tile([B, NSB * 129], BF16)  # per kj: [v_blk | ones]

            # scores + exp for one (head, kj) block-row
            pTs = []

            def _scores(h, kj, s1, chunked=False):
                pT = pTs[h]
                span = (s1 - kj) * B
                if chunked:
                    # phase-A-overlapped variant: 512-col chunks through tag B
                    # (PV's tag, idle during phase A) so the qkv psum pipeline
                    # in tag A is never paced by exp latency
                    for part in range(0, span, 512):
                        n = min(512, span - part)
                        ps_c = psum.tile([B, 512], F32, tag="B", bufs=2,
                                         name="ps_c")
                        nc.tensor.matmul(
                            ps_c[:, 0:n],
                            kT_sb[:, kj * B:(kj + 1) * B],
                            qT_sb[:, h * S + kj * B + part:
                                  h * S + kj * B + part + n],
                            start=True, stop=True,
                        )
                        if part == 0:
                            nc.vector.tensor_add(
                                ps_c[:, 0:B], ps_c[:, 0:B], dmask_sb[:]
                            )
                        nc.scalar.activation(
                            pT[:, _OFFS[kj] + part:_OFFS[kj] + part + n],
                            ps_c[:, 0:n], Exp
                        )
                    return
                ps_sc = psum.tile([B, 1024], F32, tag="A", bufs=2, name="ps_sc")
                for part in range(0, span, 512):
                    n = min(512, span - part)
                    nc.tensor.matmul(
                        ps_sc[:, part:part + n],
                        kT_sb[:, kj * B:(kj + 1) * B],
                        qT_sb[:, h * S + kj * B + part:
                              h * S + kj * B + part + n],
                        start=True, stop=True,
                    )
                # causal bias on the diagonal block
                nc.vector.tensor_add(ps_sc[:, 0:B], ps_sc[:, 0:B], dmask_sb[:])
                nc.scalar.activation(
                    pT[:, _OFFS[kj]:_OFFS[kj] + span], ps_sc[:, 0:span], Exp
                )

            # seq0/seq1 score groups overlap the back half of the qkv phase
            # (their qT/kT inputs are complete by then); seq2 runs after
            chains_done = set()
            sched = {si: [] for si in range(NSB)}
            for kj in range(0, 8):
                sched[8 + kj] = [(h, kj, 8) for h in range(HPC)]
            for kj, si in ((8, 13), (9, 14), (10, 15), (11, 15)):
                sched[si] += [(h, kj, 12) for h in range(HPC)]
            done = set()

            def _chains(scol, tp_tag="C"):
                for qi in range(scol * 4, scol * 4 + 4):
                    if qi in chains_done:
                        continue
                    _chain_qi(qi, tp_tag)

            def _chain_qi(qi, tp_tag="C", copy_act=False):
                    chains_done.add(qi)
                    s0, s1 = next(b for b in SEQ_BLOCKS if b[0] <= qi < b[1])
                    for h in range(HPC):
                        pT = pTs[h]
                        ps_pv = psum.tile([B, 129], F32, tag="B", bufs=2)
                        for kj in range(s0, qi + 1):
                            lhsT = pT[:, _OFFS[kj] + (qi - kj) * B:
                                      _OFFS[kj] + (qi - kj + 1) * B]
                            nc.tensor.matmul(
                                ps_pv[:], lhsT,
                                vaug_sb[:, kj * 129:(kj + 1) * 129],
                                start=(kj == s0), stop=(kj == qi),
                            )
                        pv_sb = work.tile([B, 129], F32, tag="pv", bufs=8)
                        nc.vector.tensor_copy(pv_sb[:], ps_pv[:])
                        rc = work.tile([B, 1], F32, tag="rc", bufs=8)
                        nc.vector.reciprocal(rc[:], pv_sb[:, 128:129])
                        at = work.tile([B, B], BF16, tag="at", bufs=8)
                        nc.vector.tensor_scalar_mul(at[:], pv_sb[:, 0:B], rc[:])
                        tp = psum.tile([B, B], BF16, tag=tp_tag, bufs=2)
                        nc.tensor.transpose(tp[:], at[:], ident_sb[:])
                        dst = attnT_sb[:, h * S + qi * B:h * S + (qi + 1) * B]
                        if copy_act:
                            nc.scalar.copy(dst, tp[:])
                        else:
                            nc.vector.tensor_copy(dst, tp[:])

            def _wo(scol, use_a=False):
                for cp in range(NCB):
                    # alternate accumulators across tags C and A (A is idle in
                    # the late passes) for a 4-deep pipeline
                    if use_a and cp % 2 == 1:
                        pso = psum.tile([B, 512], F32, tag="A", bufs=2,
                                        name="psoA")
                    elif use_a is None and cp >= 16 and cp % 2 == 1:
                        # late wo(0) groups: seq2 chains have drained tag B
                        pso = psum.tile([B, 512], F32, tag="B", bufs=2,
                                        name="psoB")
                    else:
                        pso = psum.tile([B, 512], F32, tag="C", bufs=2,
                                        name="pso")
                    for db in range(HPC):
                        nc.tensor.matmul(
                            pso[:],
                            wo_sb[:, db * DIM + cp * B:db * DIM + (cp + 1) * B],
                            attnT_sb[:, db * S + scol * 512:db * S + (scol + 1) * 512],
                            start=(db == 0), stop=(db == HPC - 1),
                        )
                    ot = work.tile([B, 512], BF16, tag="ot", bufs=8)
                    if cp % 2 == 0:
                        nc.scalar.copy(ot[:], pso[:])
                    else:
                        nc.vector.tensor_copy(ot[:], pso[:])
                    nc.sync.dma_start(
                        out=out_h[cp * B:(cp + 1) * B, scol * 512:(scol + 1) * 512],
                        in_=ot[:],
                    )

            # =========== Phase A: qkv projection + rope + transposes ===========
            for si in range(NSB):
                if si == 0:
                    xt_t = xt0_t
                elif si in early_xt:
                    xt_t = early_xt[si]
                else:
                    xt_t = work.tile([B, DIM], BF16, tag="xt", bufs=3)
                    nc.sync.dma_start(out=xt_t[:], in_=xt_h[si])
                # small rotating cos/sin tiles (dead after rope of this si)
                c_t = work.tile([B, 256], F32, tag="cs", bufs=3)
                nc.sync.dma_start(out=c_t[:], in_=cos_h[si])
                s_t = work.tile([B, 256], F32, tag="sn", bufs=3)
                nc.sync.dma_start(out=s_t[:], in_=sin_h[si])
                ps = psum.tile([B, W768], F32, tag="A", bufs=2)
                psQ = ps[:, 0:512]
                psKV = ps[:, 512:768]
                for cb in range(NCB):
                    lhsT = xt_t[:, cb * B:(cb + 1) * B]
                    nc.tensor.matmul(
                        psQ, lhsT, wqkv_sb[:, cb * W768:cb * W768 + 512],
                        start=(cb == 0), stop=(cb == NCB - 1),
                    )
                    nc.tensor.matmul(
                        psKV, lhsT,
                        wqkv_sb[:, cb * W768 + 512:cb * W768 + 768],
                        start=(cb == 0), stop=(cb == NCB - 1),
                    )

                cs = c_t[:]
                sn = s_t[:]

                # rope on q: [s, d] layout, channels interleaved (even, odd)
                q_t = work.tile([B, QW], BF16, tag="q", bufs=3)
                qe, qo = ps[:, 0:QW:2], ps[:, 1:QW:2]
                t1 = work.tile([B, 256], F32, tag="t1", bufs=2)
                t2 = work.tile([B, 256], F32, tag="t2", bufs=2)
                t3 = work.tile([B, 256], F32, tag="t3", bufs=2)
                t4 = work.tile([B, 256], F32, tag="t4", bufs=2)
                nc.vector.tensor_mul(t1[:], qe, cs)
                nc.vector.tensor_mul(t2[:], qo, sn)
                nc.vector.tensor_sub(q_t[:, 0:QW:2], t1[:], t2[:])
                nc.vector.tensor_mul(t3[:], qe, sn)
                nc.vector.tensor_mul(t4[:], qo, cs)
                nc.vector.tensor_add(q_t[:, 1:QW:2], t3[:], t4[:])

                # rope on k
                k_t = work.tile([B, KW], BF16, tag="k", bufs=3)
                ke, ko = ps[:, 512:640:2], ps[:, 513:640:2]
                c64, s64 = c_t[:, 0:64], s_t[:, 0:64]
                u1 = work.tile([B, 64], F32, tag="u1", bufs=2)
                u2 = work.tile([B, 64], F32, tag="u2", bufs=2)
                u3 = work.tile([B, 64], F32, tag="u3", bufs=2)
                u4 = work.tile([B, 64], F32, tag="u4", bufs=2)
                nc.vector.tensor_mul(u1[:], ke, c64)
                nc.vector.tensor_mul(u2[:], ko, s64)
                nc.vector.tensor_sub(k_t[:, 0:KW:2], u1[:], u2[:])
                nc.vector.tensor_mul(u3[:], ke, s64)
                nc.vector.tensor_mul(u4[:], ko, c64)
                nc.vector.tensor_add(k_t[:, 1:KW:2], u3[:], u4[:])

                # v block + ones column
                nc.scalar.copy(vaug_sb[:, si * 129:si * 129 + 128], ps[:, 640:768])
                nc.vector.memset(vaug_sb[:, si * 129 + 128:si * 129 + 129], 1.0)

                # transposes: q (4 blocks) and k (1 block)
                for h in range(HPC):
                    tp = psum.tile([B, B], BF16, tag="C", bufs=2)
                    nc.tensor.transpose(tp[:], q_t[:, h * B:(h + 1) * B], ident_sb[:])
                    dst = qT_sb[:, h * S + si * B:h * S + (si + 1) * B]
                    if h % 2 == 0:
                        nc.vector.tensor_copy(dst, tp[:])
                    else:
                        nc.scalar.copy(dst, tp[:])
                ktp = psum.tile([B, B], BF16, tag="C", bufs=2)
                nc.tensor.transpose(ktp[:], k_t[:], ident_sb[:])
                nc.vector.tensor_copy(kT_sb[:, si * B:(si + 1) * B], ktp[:])

                if si == 7:
                    for h in range(HPC):
                        pT = roam.tile([B, PT_COLS], BF16, tag="roam", bufs=4,
                                       name=f"pT{h}")
                        pTs.append(pT)
                for (h, kj, s1) in sched[si]:
                    _scores(h, kj, s1, chunked=True)
                    done.add((h, kj))
                qi_sched = {12: (0, 4), 13: (1, 5, 8), 14: (2, 6, 9),
                            15: (3, 7, 10, 11)}
                for qi in qi_sched.get(si, ()):
                    _chain_qi(qi)

            # wo reuses the wqkv slot; attnT is its own tensor so chains can
            # write it before the last qkv matmul retires
            big2 = bigp.tile([B, NCB * W768], BF16, tag="big")
            wo_sb = big2[:, 0:HPC * DIM]
            for db in range(HPC):
                nc.sync.dma_start(
                    out=wo_sb[:, db * DIM:(db + 1) * DIM],
                    in_=wo_h[db * B:(db + 1) * B, :],
                )

            _B1_TODO = [
                (h, kj, s1)
                for s0, s1 in SEQ_BLOCKS
                for kj in range(s0, s1)
                for h in range(HPC)
                if (h, kj) not in done
            ]

            # ===== Phase B2: PV + normalize, interleaved with wo per scol =====
            # chains run one scol-group ahead of the output projection so the
            # dense wo matmuls overlap the latency-bound softmax chains
            # seq0/seq1 chains already ran inside phase A; seq2 scores and
            # its chains overlap the dense wo passes
            # seq2: emit each kj's scores then immediately its qi=kj chain
            # (qi needs only exps kj'<=qi), all overlapping wo(0)'s dense work
            _wo(0, use_a=None)
            for kj in range(12, 16):
                for h in range(HPC):
                    if (h, kj) not in done:
                        _scores(h, kj, 16)
                _chain_qi(kj, tp_tag="B", copy_act=True)
            _wo(1, use_a=True)
            _wo(2, use_a=True)
            _wo(3, use_a=True)

    nc.finalize()
    return nc


def get_program():
    global _PROGRAM
    if _PROGRAM is None:
        _PROGRAM = _build_program()
    return _PROGRAM


def make_in_maps(x, cos, sin, wq, wk, wv, wo):
    bf16 = np.dtype("bfloat16") if hasattr(np, "bfloat16") else None
    import ml_dtypes
    bf16 = ml_dtypes.bfloat16

    x = np.asarray(x, np.float32)
    cos = np.asarray(cos, np.float32)
    sin = np.asarray(sin, np.float32)
    wq = np.asarray(wq, np.float32)
    wk = np.asarray(wk, np.float32)
    wv = np.asarray(wv, np.float32)
    wo = np.asarray(wo, np.float32)

    # xt[si, p, cb*B + s] = x[si*B + s, cb*B + p]
    xt = np.ascontiguousarray(
        x.reshape(NSB, B, NCB, B).transpose(0, 3, 2, 1).reshape(NSB, B, DIM)
    ).astype(bf16)
    # cos/sin tiled 4x along channels (per-head repeat), blocked by si
    cosr = np.ascontiguousarray(np.tile(cos, (1, HPC)).reshape(NSB, B, 2 * HEAD_DIM))
    sinr = np.ascontiguousarray(np.tile(sin, (1, HPC)).reshape(NSB, B, 2 * HEAD_DIM))
    # diagonal-block causal bias in scoresT layout: allow sq >= sk
    i = np.arange(B)
    dmask = np.where(i[None, :] >= i[:, None], 0.0, -30000.0).astype(np.float32)
    ident = np.eye(B, dtype=np.float32).astype(bf16)

    scale = HEAD_DIM ** -0.5
    in_maps = []
    for c in range(N_CORES):
        wq_c = (wq[:, c * QW:(c + 1) * QW] * scale).astype(bf16)
        wk_c = wk[:, c * KW:(c + 1) * KW].astype(bf16)
        wv_c = wv[:, c * KW:(c + 1) * KW].astype(bf16)
        wqkv_c = np.ascontiguousarray(
            np.concatenate([wq_c, wk_c, wv_c], axis=1)
        )
        wo_c = np.ascontiguousarray(wo[c * QW:(c + 1) * QW, :]).astype(bf16)
        in_maps.append({
            "xt": xt,
            "wqkv": wqkv_c,
            "wo": wo_c,
            "cosr": cosr,
            "sinr": sinr,
            "dmask": dmask,
            "ident": ident,
        })
    return in_maps


def combine_outputs(results):
    acc = np.zeros((DIM, S), np.float32)
    for r in results:
        acc += np.asarray(r["outp"]).astype(np.float32)
    return np.ascontiguousarray(acc.T)


def kernel(x, cos, sin, mask, wq, wk, wv, wo):
    nc = get_program()
    in_maps = make_in_maps(x, cos, sin, wq, wk, wv, wo)
    res = run_bass_kernel_spmd(nc, in_maps, core_ids=list(range(N_CORES)))
    return combine_outputs(res.results)
